# revision 34
# baseline (speedup 1.0000x reference)
"""GAT (2-head, 64-ch) + BatchNorm message passing on 8 Trainium2 cores.

Dst-node graph parallel: 12500 dst nodes/core (98 blocks x 128, in-degree
sorted so per-block edge counts are uniform across cores). Phase 0 computes
h_aug = x @ [W | W@att_src^T | W@att_dst^T] per shard into 512-byte table
rows [h(128f16) | a_src(2) | a_dst(2) | pad]; the table is AllGathered in 4
rank-quarter pieces so quartile-q gathers can start as soon as piece q
lands. Edges are routed to the dst core, bucketed per (dst block, src
quartile) padded to 128-edge chunks (uniform chunk grid across cores), and
gathered 8 chunks (1024 rows) per dma_gather on 4 SWDGE queues with
prefetch. Per chunk: a_dst expand via one PE matmul with a host-uploaded
transposed one-hot (eqT); esc/leaky-relu/exp batched per 8-chunk batch;
w-scaled values built per head on DVE/ACT; one PE matmul per chunk
accumulates numerator and denominator into the block PSUM. Block finalize
adds the self-loop (scores straight from phase-0 columns), normalizes,
applies ReLU, and accumulates BN stats; stats are AllReduced and the BN
affine applied in a final pass.
"""
import sys
sys.path.insert(0, "/opt/trn_rl_repo")
import numpy as np

N = 100_000
F = 128
H = 2
C = 64
HC = H * C
NEG_SLOPE = 0.2
BN_EPS = 1e-5
NCORES = 8
NSH_RAW = 12_500
NSH = 12_544            # 98 * 128
NB = NSH // 128         # 98
QS = 4
# block-aligned shard quarters (ranks); quartile tables are 8x these rows
QSH_P = [3200, 3200, 3072, 3072]
PS_P = [0, 3200, 6400, 9472, 12544]
P = 128
ROWW = 128              # table row width in f16 (256 B)
BCH = 16                # chunks per gather batch (2048 rows)
BROWS = BCH * P
PADVAL = 200.0
PREFETCH = 3


def _host_prep(x, edge_index, W, att_src, att_dst, bias, gamma, beta):
    src = np.asarray(edge_index[0]).astype(np.int64)
    dst = np.asarray(edge_index[1]).astype(np.int64)
    x = np.asarray(x, dtype=np.float32)
    W = np.asarray(W, dtype=np.float32)
    att_src = np.asarray(att_src, dtype=np.float32)
    att_dst = np.asarray(att_dst, dtype=np.float32)

    W_aug = np.zeros((F, 132), dtype=np.float32)
    W_aug[:, 0:HC] = W
    W_aug[:, HC:HC + 2] = np.einsum(
        "fhc,hc->fh", W.reshape(F, H, C), att_src)
    W_aug[:, HC + 2:HC + 4] = np.einsum(
        "fhc,hc->fh", W.reshape(F, H, C), att_dst)
    asrc_all = x @ W_aug[:, HC:HC + 2]                   # [N, 2] f32

    gbb = np.zeros((1, 3 * HC), dtype=np.float32)
    gbb[0, 0:HC] = np.asarray(gamma, dtype=np.float32).reshape(-1)
    gbb[0, HC:2 * HC] = np.asarray(beta, dtype=np.float32).reshape(-1)
    gbb[0, 2 * HC:] = np.asarray(bias, dtype=np.float32).reshape(-1)

    # per-core in-degree rank (degree-sorted blocks)
    orders, ranks = [], []
    for c in range(NCORES):
        m = (dst // NSH_RAW) == c
        d_loc = dst[m] - c * NSH_RAW
        deg = np.bincount(d_loc, minlength=NSH_RAW)
        order = np.argsort(-deg, kind="stable")
        rank = np.empty(NSH_RAW, dtype=np.int64)
        rank[order] = np.arange(NSH_RAW)
        orders.append(order)
        ranks.append(rank)

    ps = np.asarray(PS_P, dtype=np.int64)
    qsh = np.asarray(QSH_P, dtype=np.int64)

    # per-core edge bucketing by (dst block, src quartile)
    core_ed = []
    cnts = np.zeros((NCORES, NB, QS), dtype=np.int64)
    for c in range(NCORES):
        m = (dst // NSH_RAW) == c
        s_c = src[m]
        r_d = ranks[c][dst[m] - c * NSH_RAW]
        w = r_d // 128
        j = r_d % 128
        cs = s_c // NSH_RAW
        r_s_local = np.empty(len(s_c), dtype=np.int64)
        for c2 in range(NCORES):
            mm = cs == c2
            r_s_local[mm] = ranks[c2][s_c[mm] - c2 * NSH_RAW]
        q = (np.searchsorted(ps, r_s_local, side="right") - 1).astype(np.int64)
        idx16 = cs * qsh[q] + (r_s_local - ps[q])
        core_ed.append((w, j, q, idx16, s_c))
        np.add.at(cnts[c], (w, q), 1)

    K = ((cnts.max(axis=0) + 127) // 128).astype(np.int64)   # [NB, QS]
    SK_q = K.sum(axis=0)                                     # chunks/stream
    TOTCH = int(SK_q.sum())
    nbatch_q = [int((SK_q[q] + BCH - 1) // BCH) for q in range(QS)]
    base_wq = np.zeros((NB, QS), dtype=np.int64)             # chunk base of (w,q)
    for q in range(QS):
        base_wq[1:, q] = np.cumsum(K[:-1, q])
    off_q = np.zeros(QS, dtype=np.int64)                     # stream col offset
    off_q[1:] = np.cumsum(SK_q[:-1])

    per_core = []
    core_src = []
    for c in range(NCORES):
        core_src.append(None)
    for c in range(NCORES):
        w, j, q, idx16, s_c = core_ed[c]
        ordv = np.lexsort((idx16, j, w + NB * q))
        wq, jq, qq, iq = w[ordv], j[ordv], q[ordv], idx16[ordv]
        core_src[c] = s_c[ordv]

        import ml_dtypes
        f8 = ml_dtypes.float8_e4m3
        idx_streams = [np.zeros(nbatch_q[s] * BROWS, dtype=np.int16)
                       for s in range(QS)]
        eqT = np.zeros((128, TOTCH * 128), dtype=f8)
        eqN = np.zeros((128, TOTCH * 128), dtype=f8)
        asrcS = np.zeros((TOTCH * 128, 2), dtype=np.float16)
        src_glob = src  # global src ids (closure)
        sg = core_src[c]
        for s in range(QS):
            ms = qq == s
            ws, js, is_, gsrc = wq[ms], jq[ms], iq[ms], sg[ms]
            blo = np.searchsorted(ws, np.arange(NB))
            bhi = np.searchsorted(ws, np.arange(NB) + 1)
            for wv in range(NB):
                a, b = int(blo[wv]), int(bhi[wv])
                ne = b - a
                if ne == 0:
                    continue
                sbase = int(base_wq[wv, s]) * 128          # row in stream s
                idx_streams[s][sbase:sbase + ne] = is_[a:b].astype(np.int16)
                gch = (int(off_q[s]) + int(base_wq[wv, s])) * 128
                eqT[js[a:b], gch + np.arange(ne)] = 1.0
                # eq rows: partition = edge slot within its chunk
                esl = (gch + np.arange(ne)) % 128
                ech = (gch + np.arange(ne)) // 128
                eqN[esl, ech * 128 + js[a:b]] = 1.0
                asrcS[gch:gch + ne] = asrc_all[gsrc[a:b]].astype(np.float16)

        # [128 slot, TOTCH*2] layout: col ch*2+h
        asrc_t = np.ascontiguousarray(
            asrcS.reshape(TOTCH, 128, 2).transpose(1, 0, 2).reshape(
                128, TOTCH * 2))
        inp = {
            "xT": None,          # filled below
            "W_aug": W_aug,
            "gbb": gbb,
            "asrcS": asrc_t,
            "eqT": eqT,
            "eqN": eqN,
        }
        for s in range(QS):
            lin = idx_streams[s]
            wrapped = lin.reshape(-1, 16).T                  # [16, nb*64]
            arr = np.zeros((P, nbatch_q[s] * (BROWS // 16)), dtype=np.int16)
            for grp in range(8):
                arr[grp * 16:(grp + 1) * 16, :] = wrapped
            inp[f"idx16_{s}"] = arr

        xs = np.zeros((NSH, F), dtype=np.float32)
        xs[:NSH_RAW] = x[c * NSH_RAW:(c + 1) * NSH_RAW][orders[c]]
        inp["xT"] = np.ascontiguousarray(xs.T).astype(np.float16)
        per_core.append(inp)

    meta = dict(K=K, SK_q=SK_q, TOTCH=TOTCH, nbatch_q=nbatch_q,
                base_wq=base_wq, off_q=off_q, orders=orders)
    return per_core, meta


def _split_waits(nc, mybir, keep=1):
    """Walrus accepts at most one sem-wait on DMA/CTRL pseudo instructions;
    hoist excess waits onto InstEventSemaphore."""
    for f in nc.m.functions:
        for bb in f.blocks:
            new = []
            for ins in bb.instructions:
                si = ins.sync_info
                if si is not None and si.on_wait and len(si.on_wait) > keep:
                    for jj, wcond in enumerate(list(si.on_wait)[:-keep]):
                        w = mybir.InstEventSemaphore(
                            name=f"{ins.name}-ws{jj}", ins=[], outs=[])
                        w.engine = ins.engine
                        w.sync_info = mybir.SyncInfo(
                            on_wait=[wcond], on_update=[])
                        new.append(w)
                    ins.sync_info = mybir.SyncInfo(
                        on_wait=list(si.on_wait)[-keep:],
                        on_update=list(si.on_update))
                new.append(ins)
            bb.instructions[:] = new


def _build_program(meta, has_bias):
    import concourse.bass as bass
    import concourse.mybir as mybir
    import concourse.tile as tile
    from concourse.masks import make_identity
    from concourse.library_config import mlp as mlp_lib
    from concourse.library_overlay import lower_extended_insts

    K = meta["K"]; SK_q = meta["SK_q"]; TOTCH = meta["TOTCH"]
    nbatch_q = meta["nbatch_q"]; base_wq = meta["base_wq"]
    off_q = meta["off_q"]
    f16 = mybir.dt.float16
    f32 = mybir.dt.float32
    f8 = mybir.dt.float8e4
    i16 = mybir.dt.int16
    AF = mybir.ActivationFunctionType
    OP = mybir.AluOpType

    nc = bass.Bass(num_devices=NCORES, num_swdge_queues=QS)
    xT_in = nc.dram_tensor("xT", [F, NSH], f16, kind="ExternalInput")
    W_in = nc.dram_tensor("W_aug", [F, 132], f32, kind="ExternalInput")
    gbb_in = nc.dram_tensor("gbb", [1, 3 * HC], f32, kind="ExternalInput")
    asrc_in = nc.dram_tensor("asrcS", [P, TOTCH * 2], f16,
                             kind="ExternalInput")
    eqT_in = nc.dram_tensor("eqT", [P, TOTCH * 128], f8,
                            kind="ExternalInput")
    eqN_in = nc.dram_tensor("eqN", [P, TOTCH * 128], f8,
                            kind="ExternalInput")
    idx_in = [nc.dram_tensor(f"idx16_{q}", [P, nbatch_q[q] * (BROWS // 16)],
                             i16, kind="ExternalInput") for q in range(QS)]
    out_dram = nc.dram_tensor("out_shard", [NSH, HC], f32,
                              kind="ExternalOutput")
    import os as _os
    _dbg = bool(_os.environ.get("KERNEL_DEBUG"))
    if _dbg:
        dbg_g = nc.dram_tensor("dbg_g", [P, BCH * ROWW], f32,
                               kind="ExternalOutput")
        dbg_esc = nc.dram_tensor("dbg_esc", [P, 2 * BCH], f32,
                                 kind="ExternalOutput")
        dbg_rhs = nc.dram_tensor("dbg_rhs", [P, BCH * 130], f32,
                                 kind="ExternalOutput")
        dbg_pre = nc.dram_tensor("dbg_pre", [NSH, HC], f32,
                                 kind="ExternalOutput")
        dbg_hsh = nc.dram_tensor("dbg_hsh", [P, ROWW], f32,
                                 kind="ExternalOutput")
        dbg_hfu = nc.dram_tensor("dbg_hfu", [P, ROWW], f32,
                                 kind="ExternalOutput")

    # chunk -> block map per stream
    blockof = [np.repeat(np.arange(NB), K[:, q]) for q in range(QS)]
    # first/last chunk of each block (global over the 4 streams' chunklists)
    remaining0 = K.sum(axis=1)

    with tile.TileContext(nc) as tc:
        with tc.tile_pool(name="cst", bufs=1) as cst, \
             tc.tile_pool(name="sb", bufs=2) as sb, \
             tc.tile_pool(name="ps", bufs=1, space="PSUM") as psp, \
             tc.tile_pool(name="dram", bufs=1, space="DRAM") as dram:
            ph0_cm = tc.tile_pool(name="ph0", bufs=2)
            ph0 = ph0_cm.__enter__()

            ident = cst.tile([P, P], f16)
            make_identity(nc, ident[:])
            iota_i = cst.tile([P, P], mybir.dt.int32)
            nc.gpsimd.iota(iota_i[:], pattern=[[1, P]], channel_multiplier=0)
            iota16 = cst.tile([P, P], f16)
            nc.vector.tensor_copy(iota16[:], iota_i[:])
            ones16 = cst.tile([P, 1], f16)
            nc.vector.memset(ones16[:], 1.0)
            ones_row = cst.tile([1, P], f32)
            nc.vector.memset(ones_row[:], 1.0)
            W_f32 = cst.tile([F, 132], f32)
            nc.sync.dma_start(W_f32[:], W_in[:])
            W_sb = cst.tile([F, 132], f16)
            nc.vector.tensor_copy(W_sb[:], W_f32[:])
            gbb_sb = cst.tile([1, 3 * HC], f32)
            nc.sync.dma_start(gbb_sb[:], gbb_in[:])
            asrc_sb = cst.tile([P, TOTCH * 2], f16)
            nc.sync.dma_start(asrc_sb[:], asrc_in[:])
            idx_sb = []
            for q in range(QS):
                t = cst.tile([P, nbatch_q[q] * (BROWS // 16)], i16,
                             name=f"idxsb{q}")
                nc.sync.dma_start(t[:], idx_in[q][:])
                idx_sb.append(t)
            sc_acc = cst.tile([P, NB * 4], f16)
            out_acc = cst.tile([P, NB * HC], f16)

            nc.gpsimd.load_library(mlp_lib)

            # ---------------- phase 0: augmented h table ----------------
            h_shard_p = [dram.tile([QSH_P[p], ROWW], f16, name=f"hsh{p}")
                         for p in range(QS)]
            h_full_p = [dram.tile([NCORES * QSH_P[p], ROWW], f16,
                                  name=f"hfu{p}") for p in range(QS)]
            ag_after = {(PS_P[p + 1] - 1) // 128: p for p in range(QS)}

            nidx_regs = {}

            def reg_for(v):
                if v not in nidx_regs:
                    nidx_regs[v] = nc.gpsimd.to_reg(v)
                return nidx_regs[v]

            gtiles = {}

            def issue_gather(q, b):
                gt = sb.tile([P, BCH * ROWW], f16, tag=f"g{q}", bufs=6,
                             name=f"g{q}_{b}")
                nc.gpsimd.dma_gather(
                    out_ap=gt[:].rearrange("p (k d) -> p k d", d=ROWW),
                    in_ap=h_full_p[q][:, :],
                    idxs_ap=idx_sb[q][:, b * (BROWS // 16):
                                      (b + 1) * (BROWS // 16)],
                    num_idxs=BROWS,
                    num_idxs_reg=reg_for(BROWS),
                    elem_size=ROWW,
                    single_packet=False,
                    queue_num=q)
                gtiles[(q, b)] = gt

            next_issue = [0] * QS

            def prefetch(q, upto):
                while next_issue[q] <= min(upto, nbatch_q[q] - 1):
                    issue_gather(q, next_issue[q])
                    next_issue[q] += 1

            GRP = 14                       # blocks per phase-0 group
            for g in range(NB // GRP):
                xt = ph0.tile([P, GRP * P], f16, tag="xt", bufs=2)
                nc.sync.dma_start(
                    xt[:], xT_in[:, g * GRP * 128:(g + 1) * GRP * 128])
                h_sb = ph0.tile([P, GRP * ROWW], f16, tag="hsb", bufs=2)
                for v in range(GRP):
                    w = g * GRP + v
                    h_ps = psp.tile([P, 132], f32, tag="escp", bufs=2)
                    nc.tensor.matmul(h_ps[:], lhsT=xt[:, v * 128:
                                                      (v + 1) * 128],
                                     rhs=W_sb[:], start=True, stop=True)
                    nc.scalar.copy(
                        h_sb[:, v * ROWW:v * ROWW + HC], h_ps[:, 0:HC])
                    nc.vector.tensor_copy(sc_acc[:, 4 * w:4 * w + 4],
                                          h_ps[:, 128:132])
                # write group rows [1792g, 1792(g+1)) split by quarter piece
                r0, r1 = g * GRP * 128, (g + 1) * GRP * 128
                hv = h_sb[:].rearrange("p (v d) -> p v d", d=ROWW)
                for pc in range(QS):
                    a = max(r0, PS_P[pc])
                    bnd = min(r1, PS_P[pc + 1])
                    if a >= bnd:
                        continue
                    nc.sync.dma_start(
                        h_shard_p[pc][a - PS_P[pc]:bnd - PS_P[pc], :]
                        .rearrange("(v j) d -> j v d", j=P),
                        hv[:, (a - r0) // 128:(bnd - r0) // 128, :])
                for w in range(g * GRP, (g + 1) * GRP):
                    if w in ag_after:
                        p = ag_after[w]
                        nc.gpsimd.collective_compute(
                            "AllGather", OP.bypass,
                            replica_groups=[list(range(NCORES))],
                            ins=[h_shard_p[p][:].opt()],
                            outs=[h_full_p[p][:].opt()])
            ph0_cm.__exit__(None, None, None)
            # issue every gather upfront in strict queue rotation: the rings
            # and tile-slot waits self-regulate, keeping all 4 SWDGE queues'
            # transfers concurrently in flight
            for d in range(max(nbatch_q)):
                for q in range(QS):
                    prefetch(q, d)

            # self-loop scores for all blocks at once
            sc_v = sc_acc[:].rearrange("p (w d) -> p w d", d=4)
            esc_s = cst.tile([P, NB * 2], f32)
            nc.vector.tensor_tensor(
                out=esc_s[:].rearrange("p (w d) -> p w d", d=2),
                in0=sc_v[:, :, 0:2], in1=sc_v[:, :, 2:4], op=OP.add)
            t02_s = cst.tile([P, NB * 2], f32)
            nc.vector.tensor_scalar(out=t02_s[:], in0=esc_s[:],
                                    scalar1=NEG_SLOPE, scalar2=None,
                                    op0=OP.mult)
            nc.vector.tensor_tensor(out=esc_s[:], in0=t02_s[:],
                                    in1=esc_s[:], op=OP.max)
            expv_s = cst.tile([P, NB * 2], f32)
            nc.scalar.activation(expv_s[:], esc_s[:], AF.Exp)

            if has_bias:
                bias_ps = psp.tile([P, HC], f32, tag="escp", bufs=2)
                nc.tensor.matmul(bias_ps[:], lhsT=ones_row[:],
                                 rhs=gbb_sb[:, 2 * HC:3 * HC],
                                 start=True, stop=True)
                bias_bc = cst.tile([P, HC], f32)
                nc.vector.tensor_copy(bias_bc[:], bias_ps[:])

            stats_ps = psp.tile([1, 2 * HC], f32, tag="stats", bufs=1)
            nc.vector.memset(stats_ps[:], 0.0)

            # ---------------- main loop (batch-major) ----------------
            agg_tiles = {}           # triple t -> psum tile [P, 3*130]
            triple_left = {}
            started = set()
            remaining = remaining0.copy()
            nfin = [0]

            def agg_slice(w):
                if w not in agg_tiles:
                    agg_tiles[w] = psp.tile([P, HC + 2], f32,
                                            tag="agg", bufs=5,
                                            name=f"agg{w}")
                return agg_tiles[w], 0

            def finalize(w):
                gself = sb.tile([P, HC], f16, tag="gself", bufs=3)
                p0 = next(p for p in range(QS)
                          if PS_P[p] <= w * 128 < PS_P[p + 1])
                r0 = w * 128 - PS_P[p0]
                nc.sync.dma_start(gself[:], h_shard_p[p0][r0:r0 + 128, :])
                rhs_s = sb.tile([P, 130], f16, tag="rhss", bufs=2)
                nc.vector.tensor_scalar(
                    out=rhs_s[:, 0:C], in0=gself[:, 0:C],
                    scalar1=expv_s[:, 2 * w:2 * w + 1], scalar2=None,
                    op0=OP.mult)
                nc.scalar.activation(
                    rhs_s[:, C:HC], gself[:, C:HC], AF.Copy,
                    scale=expv_s[:, 2 * w + 1:2 * w + 2])
                nc.vector.tensor_copy(rhs_s[:, HC:HC + 2],
                                      expv_s[:, 2 * w:2 * w + 2])
                agg_t, ao = agg_slice(w)
                nc.tensor.matmul(agg_t[:, ao:ao + HC + 2], lhsT=ident[:],
                                 rhs=rhs_s[:], start=False, stop=True)
                recip = sb.tile([P, 2], f32, tag="recip", bufs=3)
                nc.vector.reciprocal(recip[:], agg_t[:, ao + HC:ao + HC + 2])
                oslice = out_acc[:, w * HC:(w + 1) * HC]
                for h in range(H):
                    if has_bias:
                        tmp = sb.tile([P, C], f32, tag="tmpb", bufs=2)
                        nc.vector.tensor_scalar(
                            out=tmp[:],
                            in0=agg_t[:, ao + C * h:ao + C * (h + 1)],
                            scalar1=recip[:, h:h + 1], scalar2=None,
                            op0=OP.mult)
                        nc.vector.tensor_tensor(
                            out=tmp[:], in0=tmp[:],
                            in1=bias_bc[:, C * h:C * (h + 1)], op=OP.add)
                        nc.vector.tensor_scalar(
                            out=oslice[:, C * h:C * (h + 1)], in0=tmp[:],
                            scalar1=0.0, scalar2=None, op0=OP.max)
                    else:
                        nc.vector.tensor_scalar(
                            out=oslice[:, C * h:C * (h + 1)],
                            in0=agg_t[:, ao + C * h:ao + C * (h + 1)],
                            scalar1=recip[:, h:h + 1], scalar2=0.0,
                            op0=OP.mult, op1=OP.max)
                agg_tiles.pop(w)
                sq_t = sb.tile([P, HC], f16, tag="sq", bufs=2)
                nc.vector.tensor_tensor(out=sq_t[:], in0=oslice, in1=oslice,
                                        op=OP.mult)
                nc.tensor.matmul(stats_ps[:, 0:HC], lhsT=ones16[:],
                                 rhs=oslice, start=False,
                                 stop=(nfin[0] == NB - 1))
                nc.tensor.matmul(stats_ps[:, HC:2 * HC], lhsT=ones16[:],
                                 rhs=sq_t[:], start=False,
                                 stop=(nfin[0] == NB - 1))
                nfin[0] += 1

            if _dbg:
                tb = sb.tile([P, ROWW], f32, tag="dbg", bufs=1)
                bsh = sb.tile([P, ROWW], f16, tag="dbgh", bufs=1)
                nc.sync.dma_start(bsh[:], h_shard_p[0][0:128, :])
                nc.vector.tensor_copy(tb[:], bsh[:])
                nc.sync.dma_start(dbg_hsh[:], tb[:])
                tb2 = sb.tile([P, ROWW], f32, tag="dbg2", bufs=1)
                bfu = sb.tile([P, ROWW], f16, tag="dbgh2", bufs=1)
                nc.sync.dma_start(bfu[:], h_full_p[0][QSH_P[0] * 1:
                                                      QSH_P[0] * 1 + 128, :])
                nc.vector.tensor_copy(tb2[:], bfu[:])
                nc.sync.dma_start(dbg_hfu[:], tb2[:])
            dbg_done = [False]

            # progress-ordered batches: by starting block, then stream
            border = sorted(
                [(q, b) for q in range(QS) for b in range(nbatch_q[q])],
                key=lambda qb: (int(blockof[qb[0]][min(qb[1] * BCH,
                                len(blockof[qb[0]]) - 1)]), qb[0]))
            for (q, b) in border:
                    G = gtiles[(q, b)]
                    nch = min(BCH, int(SK_q[q]) - b * BCH)
                    gc0 = int(off_q[q]) + b * BCH         # first chunk col
                    esc_ps = psp.tile([P, 2 * BCH], f32, tag="escp", bufs=2)
                    eqT_bt = sb.tile([P, BCH * P], f8, tag="eqt", bufs=3)
                    nc.sync.dma_start(
                        eqT_bt[:, 0:nch * 128],
                        eqT_in[:, gc0 * 128:(gc0 + nch) * 128])
                    eqN_bt = sb.tile([P, BCH * P], f8, tag="eqn", bufs=3)
                    nc.sync.dma_start(
                        eqN_bt[:, 0:nch * 128],
                        eqN_in[:, gc0 * 128:(gc0 + nch) * 128])
                    for k in range(nch):
                        ci = b * BCH + k                  # stream chunk idx
                        w = int(blockof[q][ci])
                        nc.tensor.matmul(
                            esc_ps[:, 2 * k:2 * k + 2],
                            lhsT=eqT_bt[:, k * 128:(k + 1) * 128],
                            rhs=sc_acc[:, 4 * w + 2:4 * w + 4],
                            start=True, stop=True)
                    esc_sb = sb.tile([P, 2 * BCH], f32, tag="escs", bufs=3)
                    nc.vector.tensor_tensor(
                        out=esc_sb[:, 0:2 * nch],
                        in0=esc_ps[:, 0:2 * nch],
                        in1=asrc_sb[:, gc0 * 2:(gc0 + nch) * 2], op=OP.add)
                    t02 = sb.tile([P, 2 * BCH], f32, tag="t02", bufs=3)
                    nc.vector.tensor_scalar(
                        out=t02[:, 0:2 * nch], in0=esc_sb[:, 0:2 * nch],
                        scalar1=NEG_SLOPE, scalar2=None, op0=OP.mult)
                    lr = sb.tile([P, 2 * BCH], f32, tag="lr", bufs=3)
                    nc.vector.tensor_tensor(
                        out=lr[:, 0:2 * nch], in0=t02[:, 0:2 * nch],
                        in1=esc_sb[:, 0:2 * nch], op=OP.max)
                    expv = sb.tile([P, 2 * BCH], f32, tag="expv", bufs=3)
                    nc.scalar.activation(expv[:, 0:2 * nch],
                                         lr[:, 0:2 * nch], AF.Exp)
                    rhs = sb.tile([P, BCH * 130], f16, tag="rhs", bufs=3)
                    nc.vector.tensor_copy(
                        rhs[:].rearrange(
                            "p (k d) -> p k d", d=130)[:, 0:nch, 128:130],
                        expv[:].rearrange(
                            "p (k d) -> p k d", d=2)[:, 0:nch, :])
                    for k in range(nch):
                        ci = b * BCH + k
                        w = int(blockof[q][ci])
                        nc.vector.tensor_scalar(
                            out=rhs[:, 130 * k:130 * k + C],
                            in0=G[:, ROWW * k:ROWW * k + C],
                            scalar1=expv[:, 2 * k:2 * k + 1], scalar2=None,
                            op0=OP.mult)
                        nc.scalar.activation(
                            rhs[:, 130 * k + C:130 * k + HC],
                            G[:, ROWW * k + C:ROWW * k + HC],
                            AF.Copy, scale=expv[:, 2 * k + 1:2 * k + 2])
                        agg_t, ao = agg_slice(w)
                        first = w not in started
                        started.add(w)
                        nc.tensor.matmul(
                            agg_t[:, ao:ao + HC + 2],
                            lhsT=eqN_bt[:, k * 128:(k + 1) * 128],
                            rhs=rhs[:, 130 * k:130 * (k + 1)],
                            start=first, stop=False)
                        remaining[w] -= 1
                        if remaining[w] == 0:
                            finalize(w)
                    if _dbg and q == 0 and b == 0 and not dbg_done[0]:
                        dbg_done[0] = True
                        tg = sb.tile([P, BCH * ROWW], f32, tag="dbgg", bufs=1)
                        nc.vector.tensor_copy(tg[:], G[:])
                        nc.sync.dma_start(dbg_g[:], tg[:])
                        te = sb.tile([P, 2 * BCH], f32, tag="dbge", bufs=1)
                        nc.vector.tensor_copy(te[:], esc_sb[:])
                        nc.sync.dma_start(dbg_esc[:], te[:])
                        tr = sb.tile([P, BCH * 130], f32, tag="dbgr", bufs=1)
                        nc.vector.tensor_copy(tr[:], rhs[:])
                        nc.sync.dma_start(dbg_rhs[:], tr[:])

            if _dbg:
                for w in range(NB):
                    finp = sb.tile([P, HC], f32, tag="dbgp", bufs=3)
                    nc.vector.tensor_copy(
                        finp[:], out_acc[:, w * HC:(w + 1) * HC])
                    nc.sync.dma_start(
                        dbg_pre[w * 128:(w + 1) * 128, :], finp[:])

            # ---------------- BN epilogue ----------------
            st_sb = sb.tile([1, 2 * HC], f32, tag="st", bufs=1)
            nc.vector.tensor_copy(st_sb[:], stats_ps[:])
            st_loc = dram.tile([1, 2 * HC], f32)
            st_glob = dram.tile([1, 2 * HC], f32)
            nc.sync.dma_start(st_loc[:], st_sb[:])
            nc.gpsimd.collective_compute(
                "AllReduce", OP.add,
                replica_groups=[list(range(NCORES))],
                ins=[st_loc[:].opt()], outs=[st_glob[:].opt()])
            st_g = sb.tile([1, 2 * HC], f32, tag="stg", bufs=1)
            nc.sync.dma_start(st_g[:], st_glob[:])

            sc2 = sb.tile([1, 2 * HC], f32, tag="sc2", bufs=1)
            mrow = sb.tile([1, HC], f32, tag="mrow", bufs=1)
            nc.vector.tensor_scalar(out=mrow[:], in0=st_g[:, 0:HC],
                                    scalar1=1.0 / N, scalar2=None,
                                    op0=OP.mult)
            vrow = sb.tile([1, HC], f32, tag="vrow", bufs=1)
            nc.vector.tensor_scalar(out=vrow[:], in0=st_g[:, HC:2 * HC],
                                    scalar1=1.0 / N, scalar2=None,
                                    op0=OP.mult)
            m2 = sb.tile([1, HC], f32, tag="m2", bufs=1)
            nc.vector.tensor_tensor(out=m2[:], in0=mrow[:], in1=mrow[:],
                                    op=OP.mult)
            nc.vector.tensor_tensor(out=vrow[:], in0=vrow[:], in1=m2[:],
                                    op=OP.subtract)
            nc.vector.tensor_scalar(out=vrow[:], in0=vrow[:],
                                    scalar1=BN_EPS, scalar2=None, op0=OP.add)
            rinv = sb.tile([1, HC], f32, tag="rinv", bufs=1)
            nc.vector.reciprocal(rinv[:], vrow[:])
            rstd = sb.tile([1, HC], f32, tag="rstd", bufs=1)
            nc.scalar.activation(rstd[:], rinv[:], AF.Sqrt)
            nc.vector.tensor_tensor(out=sc2[:, 0:HC], in0=gbb_sb[:, 0:HC],
                                    in1=rstd[:], op=OP.mult)
            msc = sb.tile([1, HC], f32, tag="msc", bufs=1)
            nc.vector.tensor_tensor(out=msc[:], in0=mrow[:],
                                    in1=sc2[:, 0:HC], op=OP.mult)
            nc.vector.tensor_tensor(out=sc2[:, HC:2 * HC],
                                    in0=gbb_sb[:, HC:2 * HC],
                                    in1=msc[:], op=OP.subtract)
            bc_ps = psp.tile([P, 2 * HC], f32, tag="escp", bufs=2)
            nc.tensor.matmul(bc_ps[:], lhsT=ones_row[:], rhs=sc2[:],
                             start=True, stop=True)
            bc_sb = sb.tile([P, 2 * HC], f32, tag="bc", bufs=1)
            nc.vector.tensor_copy(bc_sb[:], bc_ps[:])

            FGRP = 7
            for g in range(NB // FGRP):
                fin = sb.tile([P, FGRP * HC], f32, tag="fin", bufs=2)
                for v in range(FGRP):
                    w = g * FGRP + v
                    nc.vector.tensor_tensor(
                        out=fin[:, v * HC:(v + 1) * HC],
                        in0=out_acc[:, w * HC:(w + 1) * HC],
                        in1=bc_sb[:, 0:HC], op=OP.mult)
                    nc.vector.tensor_tensor(
                        out=fin[:, v * HC:(v + 1) * HC],
                        in0=fin[:, v * HC:(v + 1) * HC],
                        in1=bc_sb[:, HC:2 * HC], op=OP.add)
                nc.sync.dma_start(
                    out_dram[g * FGRP * 128:(g + 1) * FGRP * 128, :]
                    .rearrange("(v j) d -> j v d", j=P),
                    fin[:].rearrange("p (v d) -> p v d", d=HC))

    lower_extended_insts(nc)
    _split_waits(nc, mybir)
    return nc


_CACHE = {}


def kernel(**inputs):
    x = inputs["x"]
    edge_index = inputs["edge_index"]
    W = inputs["W"]
    att_src = inputs["att_src"]
    att_dst = inputs["att_dst"]
    bias = inputs["bias"]
    gamma = inputs["gamma"]
    beta = inputs["beta"]

    per_core, meta = _host_prep(x, edge_index, W, att_src, att_dst,
                                bias, gamma, beta)
    has_bias = bool(np.any(np.asarray(bias) != 0))

    import os as _os
    key = ("prog2", tuple(meta["K"].reshape(-1).tolist()), has_bias,
           bool(_os.environ.get("KERNEL_DEBUG")))
    if key in _CACHE:
        nc = _CACHE[key]
    else:
        nc = _build_program(meta, has_bias)
        _CACHE[key] = nc

    from concourse.bass_utils import run_bass_kernel_spmd
    res = run_bass_kernel_spmd(nc, per_core, core_ids=list(range(NCORES)))

    out = np.zeros((N, HC), dtype=np.float32)
    for c in range(NCORES):
        shard = res.results[c]["out_shard"]          # [NSH, HC] rank-ordered
        order = meta["orders"][c]
        out[c * NSH_RAW + order] = shard[:NSH_RAW]
    return out
